# revision 44
# baseline (speedup 1.0000x reference)
"""Trainium2 Bass kernel for nn_CrossAttention (MQA cross-attention + SwiGLU FF).

Reference computation (B=2, N=J=2048, D=1024, 16 heads x 64, FF 4096):
    xn = LN(x); cn = LN(context)
    q  = (xn @ Wq) * scale          (16 heads)
    k, v = split(cn @ Wkv)          (single KV head, MQA)
    out = softmax(q k^T + mask) v   -> @ Wout
    out += (silu(gate) * val) @ W2  where [val|gate] = xn @ W1

Sharding: 8 cores = 2 batches x 4 tensor-parallel shards. Each shard owns 4
query heads (Wq/Wout slices) and 1/4 of the SwiGLU FF (W1 col / W2 row
slices). K/V replicated within the batch group. Partial outputs are summed
host-side.

On-chip layout is feature-major (activations transposed host-side), so every
matmul consumes operands with the contraction dim on partitions and no
on-device transposes are needed. fp16 data, fp32 PSUM accumulation.

LayerNorm trick: per-token stats are reduced across the partition (feature)
axis with an all-ones [128,128] stationary matmul, which lands the stats
already broadcast across all 128 partitions — no separate broadcast step.
Softmax denominators ride along the attention PV matmul as an appended
all-ones column of V.
"""

from contextlib import ExitStack

import numpy as np

import concourse.bass as bass
import concourse.mybir as mybir
import concourse.tile as tile
from concourse import bacc
from concourse.bass_utils import run_bass_kernel_spmd

dt = mybir.dt
AF = mybir.ActivationFunctionType
ALU = mybir.AluOpType

B = 2
N = 2048          # query tokens per batch
J = 2048          # context tokens per batch
D = 1024          # model dim
HEADS = 16
DH = 64           # head dim
NSH = 4           # tensor-parallel shards per batch
HPC = HEADS // NSH          # heads per core (4)
QI = HPC * DH               # per-core q inner dim (256)
FF = 4 * D                  # 4096
FFS = FF // NSH             # per-core FF inner (1024)
KT = D // 128               # feature k-tiles (8)
NC = 512                    # token chunk (one PSUM bank at fp32)
NCH = N // NC               # 4 chunks
JTN = J // 128              # 16 context j-tiles
F16 = dt.float16
F32 = dt.float32
EPS = 1e-5


def _build(apply_b: bool, use_mask: bool):
    nc = bacc.Bacc("TRN2", target_bir_lowering=False, debug=False, num_devices=2 * NSH)

    tensors = dict(
        xT=nc.dram_tensor("xT", [D, N], F16, kind="ExternalInput"),
        cT=nc.dram_tensor("cT", [D, J], F16, kind="ExternalInput"),
        wq=nc.dram_tensor("wq", [D, QI], F16, kind="ExternalInput"),
        wkv=nc.dram_tensor("wkv", [D, 2 * DH], F16, kind="ExternalInput"),
        wout=nc.dram_tensor("wout", [QI, D], F16, kind="ExternalInput"),
        w1=nc.dram_tensor("w1", [D, 2 * FFS], F16, kind="ExternalInput"),
        w2=nc.dram_tensor("w2", [FFS, D], F16, kind="ExternalInput"),
        gx=nc.dram_tensor("gx", [128, KT], F32, kind="ExternalInput"),
        bx=nc.dram_tensor("bx", [128, KT], F32, kind="ExternalInput"),
        gc=nc.dram_tensor("gc", [128, KT], F32, kind="ExternalInput"),
        bc=nc.dram_tensor("bc", [128, KT], F32, kind="ExternalInput"),
        outT=nc.dram_tensor("outT", [D, N], F32, kind="ExternalOutput"),
    )
    if use_mask:
        tensors["maskT"] = nc.dram_tensor("maskT", [J, N], F16, kind="ExternalInput")

    with tile.TileContext(nc) as tc:
        with ExitStack() as ctx:
            _emit(ctx, nc, tc, tensors, apply_b, use_mask)
    nc.compile()
    return nc


def _emit(ctx, nc, tc, T, apply_b, use_mask):
    wp = ctx.enter_context(tc.tile_pool(name="weights", bufs=1))
    actp = ctx.enter_context(tc.tile_pool(name="acts", bufs=1))
    cnp = ctx.enter_context(tc.tile_pool(name="cn_hsw", bufs=1))
    smallp = ctx.enter_context(tc.tile_pool(name="small", bufs=1))
    sqp = ctx.enter_context(tc.tile_pool(name="sq", bufs=3))
    bcp = ctx.enter_context(tc.tile_pool(name="bcast", bufs=2))
    ep = ctx.enter_context(tc.tile_pool(name="exp", bufs=4))
    sgp = ctx.enter_context(tc.tile_pool(name="sg", bufs=3))
    rp = ctx.enter_context(tc.tile_pool(name="r", bufs=2))
    statp = ctx.enter_context(tc.tile_pool(name="stat", bufs=1))
    stat3p = ctx.enter_context(tc.tile_pool(name="stat3", bufs=2))
    outp = ctx.enter_context(tc.tile_pool(name="outstage", bufs=3))

    # PSUM budget (8 banks): sim 2x2 + av 2x1 + ffv 1 + ffg 1 = 8.
    # FF val/gate get dedicated banks so FF matmuls can fill PE gaps during
    # every other phase; LN borrows sim/av; kv/qT/out borrow av.
    psSim = ctx.enter_context(tc.tile_pool(name="psSim", bufs=2, space="PSUM"))
    psAv = ctx.enter_context(tc.tile_pool(name="psAv", bufs=2, space="PSUM"))
    psFv = ctx.enter_context(tc.tile_pool(name="psFv", bufs=1, space="PSUM"))
    psFg = ctx.enter_context(tc.tile_pool(name="psFg", bufs=1, space="PSUM"))

    # ---- activations first (LN-x gates everything), then weights ----
    xn_sb = actp.tile([128, KT * N], F16, tag="xn")
    cn_sb = cnp.tile([128, KT * N], F16, tag="cnhsw")
    for c in range(NCH):
        cs = slice(c * NC, (c + 1) * NC)
        for kt in range(KT):
            nc.sync.dma_start(xn_sb[:, kt * N:(kt + 1) * N][:, cs],
                              T["xT"][kt * 128:(kt + 1) * 128, :][:, cs])
    for c in range(NCH):
        cs = slice(c * NC, (c + 1) * NC)
        for kt in range(KT):
            nc.sync.dma_start(cn_sb[:, kt * N:(kt + 1) * N][:, cs],
                              T["cT"][kt * 128:(kt + 1) * 128, :][:, cs])

    gx_sb = smallp.tile([128, KT], F32, tag="gx")
    gc_sb = smallp.tile([128, KT], F32, tag="gc")
    nc.sync.dma_start(gx_sb[:], T["gx"][:])
    nc.sync.dma_start(gc_sb[:], T["gc"][:])
    bx_sb = bc_sb = None
    if apply_b:
        bx_sb = smallp.tile([128, KT], F32, tag="bx")
        bc_sb = smallp.tile([128, KT], F32, tag="bc")
        nc.sync.dma_start(bx_sb[:], T["bx"][:])
        nc.sync.dma_start(bc_sb[:], T["bc"][:])

    wkv_sb = wp.tile([128, KT * 2 * DH], F16, tag="wkv")
    wq_sb = wp.tile([128, KT * QI], F16, tag="wq")
    for kt in range(KT):
        nc.sync.dma_start(wkv_sb[:, kt * 2 * DH:(kt + 1) * 2 * DH],
                          T["wkv"][kt * 128:(kt + 1) * 128, :])
    for kt in range(KT):
        nc.sync.dma_start(wq_sb[:, kt * QI:(kt + 1) * QI],
                          T["wq"][kt * 128:(kt + 1) * 128, :])

    w1_sb = wp.tile([128, KT * 2 * FFS], F16, tag="w1")
    w2_sb = wp.tile([128, KT * D], F16, tag="w2")
    wout_sb = wp.tile([128, (QI // 128) * D], F16, tag="wout")
    for kt in range(KT):
        nc.sync.dma_start(w1_sb[:, kt * 2 * FFS:(kt + 1) * 2 * FFS],
                          T["w1"][kt * 128:(kt + 1) * 128, :])
    for kt in range(QI // 128):
        nc.sync.dma_start(wout_sb[:, kt * D:(kt + 1) * D],
                          T["wout"][kt * 128:(kt + 1) * 128, :])
    for kt in range(KT):
        nc.sync.dma_start(w2_sb[:, kt * D:(kt + 1) * D],
                          T["w2"][kt * 128:(kt + 1) * 128, :])

    ones_sb = smallp.tile([128, 128], F16, tag="ones")
    nc.vector.memset(ones_sb[:], 1.0)
    eps_sb = smallp.tile([128, 1], F32, tag="eps")
    nc.vector.memset(eps_sb[:], EPS)

    mask_sb = None
    if use_mask:
        mask_sb = smallp.tile([128, JTN * N], F16, tag="mask")
        for jt in range(JTN):
            nc.sync.dma_start(mask_sb[:, jt * N:(jt + 1) * N],
                              T["maskT"][jt * 128:(jt + 1) * 128, :])

    NC2 = 2 * NC

    def ln_pair(x_sb, g_sb, b_sb, c2):
        # LN over a 1024-token chunk pair. Per-token stats via ones[128,128]
        # stationary matmul: every output partition receives the same
        # cross-feature sum, i.e. the stats arrive pre-broadcast. Then rstd
        # via fast approximate reciprocal and a two-op apply:
        # xn = x*A + C with C = -mu*A.
        cs = slice(c2 * NC2, (c2 + 1) * NC2)
        s_ps = psSim.tile([128, NC2], F32, tag="sim")
        s2_ps = psSim.tile([128, NC2], F32, tag="sim")
        for kt in range(KT):
            xin = x_sb[:, kt * N:(kt + 1) * N][:, cs]
            sq = sqp.tile([128, NC2], F16, tag="sq")
            nc.scalar.square(sq[:], xin)
            for half in range(2):
                hs = slice(half * NC, (half + 1) * NC)
                nc.tensor.matmul(s_ps[:, hs], ones_sb[:], xin[:, hs],
                                 start=(kt == 0), stop=(kt == KT - 1))
                nc.tensor.matmul(s2_ps[:, hs], ones_sb[:], sq[:, hs],
                                 start=(kt == 0), stop=(kt == KT - 1))
        mu32 = statp.tile([128, NC2], F32, tag="mu")
        nc.vector.tensor_scalar_mul(mu32[:], s_ps[:], 1.0 / D)
        m2 = stat3p.tile([128, NC2], F32, tag="tmp")
        nc.vector.tensor_mul(m2[:], mu32[:], mu32[:])
        var = stat3p.tile([128, NC2], F32, tag="tmp")
        nc.vector.scalar_tensor_tensor(var[:], s2_ps[:], 1.0 / D, m2[:],
                                       ALU.mult, ALU.subtract)
        std = stat3p.tile([128, NC2], F32, tag="tmp")
        nc.scalar.activation(std[:], var[:], AF.Sqrt, bias=eps_sb[:])
        a32 = stat3p.tile([128, NC2], F32, tag="tmp")
        nc.vector.reciprocal_approx_fast(a32[:], std[:])
        A16 = bcp.tile([128, NC2], F16, tag="A")
        C16 = bcp.tile([128, NC2], F16, tag="C")
        nc.vector.tensor_copy(A16[:], a32[:])
        # C = -mu * A  (fused negate+mult+cast)
        nc.vector.scalar_tensor_tensor(C16[:], mu32[:], -1.0, a32[:],
                                       ALU.mult, ALU.mult)
        for kt in range(KT):
            xin = x_sb[:, kt * N:(kt + 1) * N][:, cs]
            t = sqp.tile([128, NC2], F16, tag="sq")
            nc.vector.tensor_mul(t[:], xin, A16[:])
            # the add runs on GpSimd: it is idle during LN, and taking the
            # second apply op off DVE shortens the LN critical path
            nc.gpsimd.tensor_add(xin, t[:], C16[:])
            if apply_b:
                # general ln_g/ln_b path (skipped when g==1 and b==0)
                nc.vector.tensor_scalar(xin, xin, g_sb[:, kt:kt + 1],
                                        b_sb[:, kt:kt + 1], ALU.mult, ALU.add)

    def attn_norm(h, c, av_ps, ao2_sb, odd_sb):
        # denominator: row 64 of av_ps -> f16 -> rank-1 broadcast to rows
        # 0-63 -> fast reciprocal -> scale the numerator rows.
        # Even heads land directly in ao2 rows 0-63; odd heads go to odd_sb
        # and are DMA-shifted onto partitions 64-127 of ao2 afterwards.
        cs = slice(c * NC, (c + 1) * NC)
        d16 = rp.tile([65, NC], F16, tag="d16")
        nc.vector.tensor_copy(d16[64:65, :], av_ps[DH:DH + 1, :])
        D_ps = psFg.tile([64, NC], F32, tag="ffg")
        nc.tensor.matmul(D_ps[:], ones_sb[64:65, 0:64], d16[64:65, :])
        R32 = rp.tile([64, NC], F32, tag="R32")
        nc.vector.reciprocal_approx_fast(R32[:], D_ps[:])
        dst = ao2_sb[0:64, :] if h % 2 == 0 else odd_sb
        nc.vector.tensor_mul(dst[:, (h // 2) * N:(h // 2 + 1) * N][:, cs],
                             av_ps[0:DH, :], R32[:])

    def attention_head(h, kT_sb, vb_sb, qT_sb, ao2_sb, odd_sb):
        qb = (h % 2) * 64
        qm = h // 2
        for c2 in range(NCH // 2):
            q0 = qm * N + c2 * 2 * NC
            avA = psAv.tile([DH + 1, NC], F32, tag="av")
            avB = psAv.tile([DH + 1, NC], F32, tag="av")
            for jt in range(JTN):
                # one 2-bank sim tile spanning 1024 q columns -> one wide exp
                sim_ps = psSim.tile([128, 2 * NC], F32, tag="sim")
                nc.tensor.matmul(sim_ps[:, 0:NC],
                                 kT_sb[qb:qb + 64, jt * 128:(jt + 1) * 128],
                                 qT_sb[qb:qb + 64, q0:q0 + NC])
                nc.tensor.matmul(sim_ps[:, NC:2 * NC],
                                 kT_sb[qb:qb + 64, jt * 128:(jt + 1) * 128],
                                 qT_sb[qb:qb + 64, q0 + NC:q0 + 2 * NC])
                if use_mask:
                    nc.vector.tensor_add(
                        sim_ps[:], sim_ps[:],
                        mask_sb[:, jt * N:(jt + 1) * N][:, c2 * 2 * NC:
                                                        (c2 + 1) * 2 * NC])
                e = ep.tile([128, 2 * NC], F16, tag="e")
                nc.scalar.activation(e[:], sim_ps[:], AF.Exp)
                vb = vb_sb[:, jt * (DH + 1):(jt + 1) * (DH + 1)]
                nc.tensor.matmul(avA[:], vb, e[:, 0:NC],
                                 start=(jt == 0), stop=(jt == JTN - 1))
                nc.tensor.matmul(avB[:], vb, e[:, NC:2 * NC],
                                 start=(jt == 0), stop=(jt == JTN - 1))
            attn_norm(h, 2 * c2, avA, ao2_sb, odd_sb)
            attn_norm(h, 2 * c2 + 1, avB, ao2_sb, odd_sb)
            warmers(1, pool=psAv, tag="av")

    def ff_block(m, hsw_sb):
        for c in range(NCH):
            cs = slice(c * NC, (c + 1) * NC)
            val_ps = psFv.tile([128, NC], F32, tag="ffv")
            gate_ps = psFg.tile([128, NC], F32, tag="ffg")
            for kt in range(KT):
                xin = xn_sb[:, kt * N:(kt + 1) * N][:, cs]
                nc.tensor.matmul(
                    val_ps[:],
                    w1_sb[:, kt * 2 * FFS + m * 128:kt * 2 * FFS + (m + 1) * 128],
                    xin, start=(kt == 0), stop=(kt == KT - 1))
                nc.tensor.matmul(
                    gate_ps[:],
                    w1_sb[:, kt * 2 * FFS + FFS + m * 128:
                          kt * 2 * FFS + FFS + (m + 1) * 128],
                    xin, start=(kt == 0), stop=(kt == KT - 1))
            sg = sgp.tile([128, NC], F16, tag="sg")
            nc.scalar.activation(sg[:], gate_ps[:], AF.Silu)
            nc.vector.tensor_mul(hsw_sb[:, m * N:(m + 1) * N][:, cs],
                                 val_ps[:], sg[:])

    # kT is duplicated onto partitions 64-127 so sim matmuls for heads at
    # q-row base 64 have matching lhsT/rhs base partitions.
    kT_sb = actp.tile([128, J], F16, tag="kT")
    vb_sb = actp.tile([128, JTN * (DH + 1)], F16, tag="vb")
    qT_sb = actp.tile([128, (QI // 128) * N], F16, tag="qT")

    def kv_chunk(c2):
        for c in range(2 * c2, 2 * c2 + 2):
            cs = slice(c * NC, (c + 1) * NC)
            k_ps = psAv.tile([64, NC], F32, tag="av")
            for kt in range(KT):
                nc.tensor.matmul(k_ps[:],
                                 wkv_sb[:, kt * 2 * DH:kt * 2 * DH + DH],
                                 cn_sb[:, kt * J:(kt + 1) * J][:, cs],
                                 start=(kt == 0), stop=(kt == KT - 1))
            nc.scalar.copy(kT_sb[0:64, cs], k_ps[:])
        for jt in range(c2 * JTN // 2, (c2 + 1) * JTN // 2):
            v_ps = psAv.tile([128, DH], F32, tag="av")
            for kt in range(KT):
                nc.tensor.matmul(
                    v_ps[:],
                    cn_sb[:, kt * J:(kt + 1) * J][:, jt * 128:(jt + 1) * 128],
                    wkv_sb[:, kt * 2 * DH + DH:(kt + 1) * 2 * DH],
                    start=(kt == 0), stop=(kt == KT - 1))
            nc.scalar.copy(vb_sb[:, jt * (DH + 1):jt * (DH + 1) + DH], v_ps[:])

    def qT_chunk(c2):
        for m in range(QI // 128):
            for c in range(2 * c2, 2 * c2 + 2):
                cs = slice(c * NC, (c + 1) * NC)
                q_ps = psAv.tile([128, NC], F32, tag="av")
                for kt in range(KT):
                    nc.tensor.matmul(
                        q_ps[:],
                        wq_sb[:, kt * QI + m * 128:kt * QI + (m + 1) * 128],
                        xn_sb[:, kt * N:(kt + 1) * N][:, cs],
                        start=(kt == 0), stop=(kt == KT - 1))
                nc.scalar.copy(qT_sb[:, m * N:(m + 1) * N][:, cs], q_ps[:])

    warm_n = [0]

    def warmers(k, pool=None, tag="ffv"):
        # tiny always-ready matmuls the scheduler slots into PE gaps; they
        # keep the HAM activity window non-idle so the PE clock stays at 2.4
        for _ in range(k):
            w_ps = (pool or psFv).tile([128, 64], F32, tag=tag)
            nc.tensor.matmul(w_ps[:], ones_sb[:], ones_sb[:, 0:64])
            warm_n[0] += 1
            i = warm_n[0] % 2
            nc.vector.tensor_copy(warm_sb[0:1, i:i + 1], w_ps[0:1, 0:1])

    warm_sb = smallp.tile([1, 2], F32, tag="warm")

    with nc.allow_low_precision("fp16 data path; all contractions accumulate fp32 in PSUM"):
        with nc.named_scope("ln"):
            nc.vector.memset(vb_sb[:], 1.0)
            warmers(8)  # trigger the HAM un-throttle right at kernel start
            for c2 in range(NCH // 2):
                ln_pair(xn_sb, gx_sb, bx_sb, c2)
                warmers(2)
                ln_pair(cn_sb, gc_sb, bc_sb, c2)
                warmers(2)
                kv_chunk(c2)
                qT_chunk(c2)
                warmers(2)
            nc.sync.dma_start(kT_sb[64:128, :], kT_sb[0:64, :])

        # ---- attention + FF, interleaved so FF matmuls fill PE gaps while
        # ---- ACT runs exp and DVE normalizes ----
        # ao2 packs head pairs [2m, 2m+1] onto partitions [0-63, 64-127] so the
        # Wout contraction runs as 2 full K=128 steps.
        ao2_sb = actp.tile([128, (QI // 128) * N], F16, tag="ao")
        odd_sb = actp.tile([64, (QI // 128) * N], F16, tag="aoodd")
        hsw_sb = cnp.tile([128, KT * N], F16, tag="cnhsw")
        with nc.named_scope("attn_ff"):
            ff_block(0, hsw_sb)
            for h in range(HPC):
                attention_head(h, kT_sb, vb_sb, qT_sb, ao2_sb, odd_sb)
                ff_block(2 * h + 1, hsw_sb)
                if h < HPC - 1:
                    ff_block(2 * h + 2, hsw_sb)
            nc.sync.dma_start(ao2_sb[64:128, :], odd_sb[:])

        # ---- out^T = Wout_s^T ao + W2_s^T hsw  (shared accumulation) ----
        with nc.named_scope("out"):
            for m in range(D // 128):
                for c in range(NCH):
                    cs = slice(c * NC, (c + 1) * NC)
                    o_ps = psAv.tile([128, NC], F32, tag="av")
                    for kt in range(QI // 128):
                        nc.tensor.matmul(
                            o_ps[:],
                            wout_sb[:, kt * D + m * 128:kt * D + (m + 1) * 128],
                            ao2_sb[:, kt * N:(kt + 1) * N][:, cs],
                            start=(kt == 0), stop=False)
                    for kt in range(KT):
                        nc.tensor.matmul(
                            o_ps[:],
                            w2_sb[:, kt * D + m * 128:kt * D + (m + 1) * 128],
                            hsw_sb[:, kt * N:(kt + 1) * N][:, cs],
                            start=False, stop=(kt == KT - 1))
                    o_sb = outp.tile([128, NC], F32, tag="o")
                    nc.vector.tensor_copy(o_sb[:], o_ps[:])
                    nc.sync.dma_start(T["outT"][m * 128:(m + 1) * 128, :][:, cs],
                                      o_sb[:])
                warmers(1, pool=psFv, tag="ffv")


_NC_CACHE = {}
_LAST_RES = None


def _get_nc(apply_b: bool, use_mask: bool):
    key = (apply_b, use_mask)
    if key not in _NC_CACHE:
        _NC_CACHE[key] = _build(apply_b, use_mask)
    return _NC_CACHE[key]


def kernel(x, context, mask, ln_g, ln_b, cln_g, cln_b, Wq, Wkv, Wout, W1, W2):
    global _LAST_RES
    x = np.asarray(x, np.float32)
    context = np.asarray(context, np.float32)
    mask = np.asarray(mask, np.float32)
    ln_g, ln_b = np.asarray(ln_g, np.float32), np.asarray(ln_b, np.float32)
    cln_g, cln_b = np.asarray(cln_g, np.float32), np.asarray(cln_b, np.float32)
    Wq, Wkv, Wout = (np.asarray(Wq, np.float32), np.asarray(Wkv, np.float32),
                     np.asarray(Wout, np.float32))
    W1, W2 = np.asarray(W1, np.float32), np.asarray(W2, np.float32)

    scale = DH ** -0.5
    use_mask = bool(np.any(mask))
    apply_b = bool(np.any(ln_b) or np.any(cln_b)
                   or np.any(ln_g != 1) or np.any(cln_g != 1))

    xT = [np.ascontiguousarray(x[b].T).astype(np.float16) for b in range(B)]
    cT = [np.ascontiguousarray(context[b].T).astype(np.float16) for b in range(B)]
    mT = [np.ascontiguousarray(mask[b].T).astype(np.float16) for b in range(B)] \
        if use_mask else None
    wkv16 = Wkv.astype(np.float16)
    pack = lambda v: np.ascontiguousarray(v.reshape(KT, 128).T).astype(np.float32)
    gxp, bxp, gcp, bcp_ = pack(ln_g), pack(ln_b), pack(cln_g), pack(cln_b)

    in_maps = []
    for core in range(B * NSH):
        bi, s = core // NSH, core % NSH
        m = {
            "xT": xT[bi],
            "cT": cT[bi],
            "wq": np.ascontiguousarray(
                Wq[:, s * QI:(s + 1) * QI] * scale).astype(np.float16),
            "wkv": wkv16,
            "wout": np.ascontiguousarray(Wout[s * QI:(s + 1) * QI, :]).astype(np.float16),
            "w1": np.ascontiguousarray(np.concatenate(
                [W1[:, s * FFS:(s + 1) * FFS],
                 W1[:, FF + s * FFS:FF + (s + 1) * FFS]], axis=1)).astype(np.float16),
            "w2": np.ascontiguousarray(W2[s * FFS:(s + 1) * FFS, :]).astype(np.float16),
            "gx": gxp, "bx": bxp, "gc": gcp, "bc": bcp_,
        }
        if use_mask:
            m["maskT"] = mT[bi]
        in_maps.append(m)

    nc = _get_nc(apply_b, use_mask)
    res = run_bass_kernel_spmd(nc, in_maps, core_ids=list(range(B * NSH)))
    _LAST_RES = res

    out = np.zeros((B, N, D), np.float32)
    for core in range(B * NSH):
        out[core // NSH] += res.results[core]["outT"].T
    return out


# revision 45
# speedup vs baseline: 1.0818x; 1.0818x over previous
"""Trainium2 Bass kernel for nn_CrossAttention (MQA cross-attention + SwiGLU FF).

Reference computation (B=2, N=J=2048, D=1024, 16 heads x 64, FF 4096):
    xn = LN(x); cn = LN(context)
    q  = (xn @ Wq) * scale          (16 heads)
    k, v = split(cn @ Wkv)          (single KV head, MQA)
    out = softmax(q k^T + mask) v   -> @ Wout
    out += (silu(gate) * val) @ W2  where [val|gate] = xn @ W1

Sharding: 8 cores = 2 batches x 4 tensor-parallel shards. Each shard owns 4
query heads (Wq/Wout slices) and 1/4 of the SwiGLU FF (W1 col / W2 row
slices). K/V replicated within the batch group. Partial outputs are summed
host-side.

On-chip layout is feature-major (activations transposed host-side), so every
matmul consumes operands with the contraction dim on partitions and no
on-device transposes are needed. fp16 data, fp32 PSUM accumulation.

LayerNorm trick: per-token stats are reduced across the partition (feature)
axis with an all-ones [128,128] stationary matmul, which lands the stats
already broadcast across all 128 partitions — no separate broadcast step.
Softmax denominators ride along the attention PV matmul as an appended
all-ones column of V.
"""

from contextlib import ExitStack

import numpy as np

import concourse.bass as bass
import concourse.mybir as mybir
import concourse.tile as tile
from concourse import bacc
from concourse.bass_utils import run_bass_kernel_spmd

dt = mybir.dt
AF = mybir.ActivationFunctionType
ALU = mybir.AluOpType

B = 2
N = 2048          # query tokens per batch
J = 2048          # context tokens per batch
D = 1024          # model dim
HEADS = 16
DH = 64           # head dim
NSH = 4           # tensor-parallel shards per batch
HPC = HEADS // NSH          # heads per core (4)
QI = HPC * DH               # per-core q inner dim (256)
FF = 4 * D                  # 4096
FFS = FF // NSH             # per-core FF inner (1024)
KT = D // 128               # feature k-tiles (8)
NC = 512                    # token chunk (one PSUM bank at fp32)
NCH = N // NC               # 4 chunks
JTN = J // 128              # 16 context j-tiles
F16 = dt.float16
F32 = dt.float32
EPS = 1e-5


def _build(apply_b: bool, use_mask: bool):
    nc = bacc.Bacc("TRN2", target_bir_lowering=False, debug=False, num_devices=2 * NSH)

    tensors = dict(
        xT=nc.dram_tensor("xT", [D, N], F16, kind="ExternalInput"),
        cT=nc.dram_tensor("cT", [D, J], F16, kind="ExternalInput"),
        wq=nc.dram_tensor("wq", [D, QI], F16, kind="ExternalInput"),
        wkv=nc.dram_tensor("wkv", [D, 2 * DH], F16, kind="ExternalInput"),
        wout=nc.dram_tensor("wout", [QI, D], F16, kind="ExternalInput"),
        w1=nc.dram_tensor("w1", [D, 2 * FFS], F16, kind="ExternalInput"),
        w2=nc.dram_tensor("w2", [FFS, D], F16, kind="ExternalInput"),
        gx=nc.dram_tensor("gx", [128, KT], F32, kind="ExternalInput"),
        bx=nc.dram_tensor("bx", [128, KT], F32, kind="ExternalInput"),
        gc=nc.dram_tensor("gc", [128, KT], F32, kind="ExternalInput"),
        bc=nc.dram_tensor("bc", [128, KT], F32, kind="ExternalInput"),
        outT=nc.dram_tensor("outT", [D, N], F32, kind="ExternalOutput"),
    )
    if use_mask:
        tensors["maskT"] = nc.dram_tensor("maskT", [J, N], F16, kind="ExternalInput")

    with tile.TileContext(nc) as tc:
        with ExitStack() as ctx:
            _emit(ctx, nc, tc, tensors, apply_b, use_mask)
    nc.compile()
    return nc


def _emit(ctx, nc, tc, T, apply_b, use_mask):
    wp = ctx.enter_context(tc.tile_pool(name="weights", bufs=1))
    actp = ctx.enter_context(tc.tile_pool(name="acts", bufs=1))
    cnp = ctx.enter_context(tc.tile_pool(name="cn_hsw", bufs=1))
    smallp = ctx.enter_context(tc.tile_pool(name="small", bufs=1))
    sqp = ctx.enter_context(tc.tile_pool(name="sq", bufs=3))
    bcp = ctx.enter_context(tc.tile_pool(name="bcast", bufs=2))
    ep = ctx.enter_context(tc.tile_pool(name="exp", bufs=4))
    sgp = ctx.enter_context(tc.tile_pool(name="sg", bufs=3))
    rp = ctx.enter_context(tc.tile_pool(name="r", bufs=2))
    statp = ctx.enter_context(tc.tile_pool(name="stat", bufs=1))
    stat3p = ctx.enter_context(tc.tile_pool(name="stat3", bufs=2))
    outp = ctx.enter_context(tc.tile_pool(name="outstage", bufs=3))

    # PSUM budget (8 banks): sim 2x2 + av 2x1 + ffv 1 + ffg 1 = 8.
    # FF val/gate get dedicated banks so FF matmuls can fill PE gaps during
    # every other phase; LN borrows sim/av; kv/qT/out borrow av.
    psSim = ctx.enter_context(tc.tile_pool(name="psSim", bufs=2, space="PSUM"))
    psAv = ctx.enter_context(tc.tile_pool(name="psAv", bufs=2, space="PSUM"))
    psFv = ctx.enter_context(tc.tile_pool(name="psFv", bufs=1, space="PSUM"))
    psFg = ctx.enter_context(tc.tile_pool(name="psFg", bufs=1, space="PSUM"))

    # ---- activations first (LN-x gates everything), then weights ----
    xn_sb = actp.tile([128, KT * N], F16, tag="xn")
    cn_sb = cnp.tile([128, KT * N], F16, tag="cnhsw")
    for c in range(NCH):
        cs = slice(c * NC, (c + 1) * NC)
        for kt in range(KT):
            nc.sync.dma_start(xn_sb[:, kt * N:(kt + 1) * N][:, cs],
                              T["xT"][kt * 128:(kt + 1) * 128, :][:, cs])
    for c in range(NCH):
        cs = slice(c * NC, (c + 1) * NC)
        for kt in range(KT):
            nc.sync.dma_start(cn_sb[:, kt * N:(kt + 1) * N][:, cs],
                              T["cT"][kt * 128:(kt + 1) * 128, :][:, cs])

    gx_sb = smallp.tile([128, KT], F32, tag="gx")
    gc_sb = smallp.tile([128, KT], F32, tag="gc")
    nc.sync.dma_start(gx_sb[:], T["gx"][:])
    nc.sync.dma_start(gc_sb[:], T["gc"][:])
    bx_sb = bc_sb = None
    if apply_b:
        bx_sb = smallp.tile([128, KT], F32, tag="bx")
        bc_sb = smallp.tile([128, KT], F32, tag="bc")
        nc.sync.dma_start(bx_sb[:], T["bx"][:])
        nc.sync.dma_start(bc_sb[:], T["bc"][:])

    wkv_sb = wp.tile([128, KT * 2 * DH], F16, tag="wkv")
    wq_sb = wp.tile([128, KT * QI], F16, tag="wq")
    for kt in range(KT):
        nc.sync.dma_start(wkv_sb[:, kt * 2 * DH:(kt + 1) * 2 * DH],
                          T["wkv"][kt * 128:(kt + 1) * 128, :])
    for kt in range(KT):
        nc.sync.dma_start(wq_sb[:, kt * QI:(kt + 1) * QI],
                          T["wq"][kt * 128:(kt + 1) * 128, :])

    w1_sb = wp.tile([128, KT * 2 * FFS], F16, tag="w1")
    w2_sb = wp.tile([128, KT * D], F16, tag="w2")
    wout_sb = wp.tile([128, (QI // 128) * D], F16, tag="wout")
    for kt in range(KT):
        nc.sync.dma_start(w1_sb[:, kt * 2 * FFS:(kt + 1) * 2 * FFS],
                          T["w1"][kt * 128:(kt + 1) * 128, :])
    for kt in range(QI // 128):
        nc.sync.dma_start(wout_sb[:, kt * D:(kt + 1) * D],
                          T["wout"][kt * 128:(kt + 1) * 128, :])
    for kt in range(KT):
        nc.sync.dma_start(w2_sb[:, kt * D:(kt + 1) * D],
                          T["w2"][kt * 128:(kt + 1) * 128, :])

    ones_sb = smallp.tile([128, 128], F16, tag="ones")
    nc.vector.memset(ones_sb[:], 1.0)
    eps_sb = smallp.tile([128, 1], F32, tag="eps")
    nc.vector.memset(eps_sb[:], EPS)

    mask_sb = None
    if use_mask:
        mask_sb = smallp.tile([128, JTN * N], F16, tag="mask")
        for jt in range(JTN):
            nc.sync.dma_start(mask_sb[:, jt * N:(jt + 1) * N],
                              T["maskT"][jt * 128:(jt + 1) * 128, :])

    NC2 = 2 * NC

    def ln_pair(x_sb, g_sb, b_sb, c2):
        # LN over a 1024-token chunk pair. Per-token stats via ones[128,128]
        # stationary matmul: every output partition receives the same
        # cross-feature sum, i.e. the stats arrive pre-broadcast. Then rstd
        # via fast approximate reciprocal and a two-op apply:
        # xn = x*A + C with C = -mu*A.
        cs = slice(c2 * NC2, (c2 + 1) * NC2)
        s_ps = psSim.tile([128, NC2], F32, tag="sim")
        s2_ps = psSim.tile([128, NC2], F32, tag="sim")
        for kt in range(KT):
            xin = x_sb[:, kt * N:(kt + 1) * N][:, cs]
            sq = sqp.tile([128, NC2], F16, tag="sq")
            nc.scalar.square(sq[:], xin)
            for half in range(2):
                hs = slice(half * NC, (half + 1) * NC)
                nc.tensor.matmul(s_ps[:, hs], ones_sb[:], xin[:, hs],
                                 start=(kt == 0), stop=(kt == KT - 1))
                nc.tensor.matmul(s2_ps[:, hs], ones_sb[:], sq[:, hs],
                                 start=(kt == 0), stop=(kt == KT - 1))
        mu32 = statp.tile([128, NC2], F32, tag="mu")
        nc.vector.tensor_scalar_mul(mu32[:], s_ps[:], 1.0 / D)
        m2 = stat3p.tile([128, NC2], F32, tag="tmp")
        nc.vector.tensor_mul(m2[:], mu32[:], mu32[:])
        var = stat3p.tile([128, NC2], F32, tag="tmp")
        nc.vector.scalar_tensor_tensor(var[:], s2_ps[:], 1.0 / D, m2[:],
                                       ALU.mult, ALU.subtract)
        std = stat3p.tile([128, NC2], F32, tag="tmp")
        nc.scalar.activation(std[:], var[:], AF.Sqrt, bias=eps_sb[:])
        a32 = stat3p.tile([128, NC2], F32, tag="tmp")
        nc.vector.reciprocal_approx_fast(a32[:], std[:])
        A16 = bcp.tile([128, NC2], F16, tag="A")
        C16 = bcp.tile([128, NC2], F16, tag="C")
        nc.vector.tensor_copy(A16[:], a32[:])
        # C = -mu * A  (fused negate+mult+cast)
        nc.vector.scalar_tensor_tensor(C16[:], mu32[:], -1.0, a32[:],
                                       ALU.mult, ALU.mult)
        for kt in range(KT):
            xin = x_sb[:, kt * N:(kt + 1) * N][:, cs]
            t = sqp.tile([128, NC2], F16, tag="sq")
            nc.vector.tensor_mul(t[:], xin, A16[:])
            nc.vector.tensor_add(xin, t[:], C16[:])
            if apply_b:
                # general ln_g/ln_b path (skipped when g==1 and b==0)
                nc.vector.tensor_scalar(xin, xin, g_sb[:, kt:kt + 1],
                                        b_sb[:, kt:kt + 1], ALU.mult, ALU.add)

    def attn_norm(h, c, av_ps, ao2_sb, odd_sb):
        # denominator: row 64 of av_ps -> f16 -> rank-1 broadcast to rows
        # 0-63 -> fast reciprocal -> scale the numerator rows.
        # Even heads land directly in ao2 rows 0-63; odd heads go to odd_sb
        # and are DMA-shifted onto partitions 64-127 of ao2 afterwards.
        cs = slice(c * NC, (c + 1) * NC)
        d16 = rp.tile([65, NC], F16, tag="d16")
        nc.vector.tensor_copy(d16[64:65, :], av_ps[DH:DH + 1, :])
        D_ps = psFg.tile([64, NC], F32, tag="ffg")
        nc.tensor.matmul(D_ps[:], ones_sb[64:65, 0:64], d16[64:65, :])
        R32 = rp.tile([64, NC], F32, tag="R32")
        nc.vector.reciprocal_approx_fast(R32[:], D_ps[:])
        dst = ao2_sb[0:64, :] if h % 2 == 0 else odd_sb
        nc.vector.tensor_mul(dst[:, (h // 2) * N:(h // 2 + 1) * N][:, cs],
                             av_ps[0:DH, :], R32[:])

    def attention_head(h, kT_sb, vb_sb, qT_sb, ao2_sb, odd_sb):
        qb = (h % 2) * 64
        qm = h // 2
        for c2 in range(NCH // 2):
            q0 = qm * N + c2 * 2 * NC
            avA = psAv.tile([DH + 1, NC], F32, tag="av")
            avB = psAv.tile([DH + 1, NC], F32, tag="av")
            for jt in range(JTN):
                # one 2-bank sim tile spanning 1024 q columns -> one wide exp
                sim_ps = psSim.tile([128, 2 * NC], F32, tag="sim")
                nc.tensor.matmul(sim_ps[:, 0:NC],
                                 kT_sb[qb:qb + 64, jt * 128:(jt + 1) * 128],
                                 qT_sb[qb:qb + 64, q0:q0 + NC])
                nc.tensor.matmul(sim_ps[:, NC:2 * NC],
                                 kT_sb[qb:qb + 64, jt * 128:(jt + 1) * 128],
                                 qT_sb[qb:qb + 64, q0 + NC:q0 + 2 * NC])
                if use_mask:
                    nc.vector.tensor_add(
                        sim_ps[:], sim_ps[:],
                        mask_sb[:, jt * N:(jt + 1) * N][:, c2 * 2 * NC:
                                                        (c2 + 1) * 2 * NC])
                e = ep.tile([128, 2 * NC], F16, tag="e")
                nc.scalar.activation(e[:], sim_ps[:], AF.Exp)
                vb = vb_sb[:, jt * (DH + 1):(jt + 1) * (DH + 1)]
                nc.tensor.matmul(avA[:], vb, e[:, 0:NC],
                                 start=(jt == 0), stop=(jt == JTN - 1))
                nc.tensor.matmul(avB[:], vb, e[:, NC:2 * NC],
                                 start=(jt == 0), stop=(jt == JTN - 1))
            attn_norm(h, 2 * c2, avA, ao2_sb, odd_sb)
            attn_norm(h, 2 * c2 + 1, avB, ao2_sb, odd_sb)
            warmers(1, pool=psAv, tag="av")

    def ff_block(m, hsw_sb):
        for c in range(NCH):
            cs = slice(c * NC, (c + 1) * NC)
            val_ps = psFv.tile([128, NC], F32, tag="ffv")
            gate_ps = psFg.tile([128, NC], F32, tag="ffg")
            for kt in range(KT):
                xin = xn_sb[:, kt * N:(kt + 1) * N][:, cs]
                nc.tensor.matmul(
                    val_ps[:],
                    w1_sb[:, kt * 2 * FFS + m * 128:kt * 2 * FFS + (m + 1) * 128],
                    xin, start=(kt == 0), stop=(kt == KT - 1))
                nc.tensor.matmul(
                    gate_ps[:],
                    w1_sb[:, kt * 2 * FFS + FFS + m * 128:
                          kt * 2 * FFS + FFS + (m + 1) * 128],
                    xin, start=(kt == 0), stop=(kt == KT - 1))
            sg = sgp.tile([128, NC], F16, tag="sg")
            nc.scalar.activation(sg[:], gate_ps[:], AF.Silu)
            nc.vector.tensor_mul(hsw_sb[:, m * N:(m + 1) * N][:, cs],
                                 val_ps[:], sg[:])

    # kT is duplicated onto partitions 64-127 so sim matmuls for heads at
    # q-row base 64 have matching lhsT/rhs base partitions.
    kT_sb = actp.tile([128, J], F16, tag="kT")
    vb_sb = actp.tile([128, JTN * (DH + 1)], F16, tag="vb")
    qT_sb = actp.tile([128, (QI // 128) * N], F16, tag="qT")

    def kv_chunk(c2):
        for c in range(2 * c2, 2 * c2 + 2):
            cs = slice(c * NC, (c + 1) * NC)
            k_ps = psAv.tile([64, NC], F32, tag="av")
            for kt in range(KT):
                nc.tensor.matmul(k_ps[:],
                                 wkv_sb[:, kt * 2 * DH:kt * 2 * DH + DH],
                                 cn_sb[:, kt * J:(kt + 1) * J][:, cs],
                                 start=(kt == 0), stop=(kt == KT - 1))
            nc.scalar.copy(kT_sb[0:64, cs], k_ps[:])
        for jt in range(c2 * JTN // 2, (c2 + 1) * JTN // 2):
            v_ps = psAv.tile([128, DH], F32, tag="av")
            for kt in range(KT):
                nc.tensor.matmul(
                    v_ps[:],
                    cn_sb[:, kt * J:(kt + 1) * J][:, jt * 128:(jt + 1) * 128],
                    wkv_sb[:, kt * 2 * DH + DH:(kt + 1) * 2 * DH],
                    start=(kt == 0), stop=(kt == KT - 1))
            nc.scalar.copy(vb_sb[:, jt * (DH + 1):jt * (DH + 1) + DH], v_ps[:])

    def qT_chunk(c2):
        for m in range(QI // 128):
            for c in range(2 * c2, 2 * c2 + 2):
                cs = slice(c * NC, (c + 1) * NC)
                q_ps = psAv.tile([128, NC], F32, tag="av")
                for kt in range(KT):
                    nc.tensor.matmul(
                        q_ps[:],
                        wq_sb[:, kt * QI + m * 128:kt * QI + (m + 1) * 128],
                        xn_sb[:, kt * N:(kt + 1) * N][:, cs],
                        start=(kt == 0), stop=(kt == KT - 1))
                nc.scalar.copy(qT_sb[:, m * N:(m + 1) * N][:, cs], q_ps[:])

    warm_n = [0]

    def warmers(k, pool=None, tag="ffv"):
        # tiny always-ready matmuls the scheduler slots into PE gaps; they
        # keep the HAM activity window non-idle so the PE clock stays at 2.4
        for _ in range(k):
            w_ps = (pool or psFv).tile([128, 64], F32, tag=tag)
            nc.tensor.matmul(w_ps[:], ones_sb[:], ones_sb[:, 0:64])
            warm_n[0] += 1
            i = warm_n[0] % 2
            nc.vector.tensor_copy(warm_sb[0:1, i:i + 1], w_ps[0:1, 0:1])

    warm_sb = smallp.tile([1, 2], F32, tag="warm")

    with nc.allow_low_precision("fp16 data path; all contractions accumulate fp32 in PSUM"):
        with nc.named_scope("ln"):
            nc.vector.memset(vb_sb[:], 1.0)
            warmers(8)  # trigger the HAM un-throttle right at kernel start
            for c2 in range(NCH // 2):
                ln_pair(xn_sb, gx_sb, bx_sb, c2)
                warmers(2)
                ln_pair(cn_sb, gc_sb, bc_sb, c2)
                warmers(2)
                kv_chunk(c2)
                qT_chunk(c2)
                warmers(2)
            nc.sync.dma_start(kT_sb[64:128, :], kT_sb[0:64, :])

        # ---- attention + FF, interleaved so FF matmuls fill PE gaps while
        # ---- ACT runs exp and DVE normalizes ----
        # ao2 packs head pairs [2m, 2m+1] onto partitions [0-63, 64-127] so the
        # Wout contraction runs as 2 full K=128 steps.
        ao2_sb = actp.tile([128, (QI // 128) * N], F16, tag="ao")
        odd_sb = actp.tile([64, (QI // 128) * N], F16, tag="aoodd")
        hsw_sb = cnp.tile([128, KT * N], F16, tag="cnhsw")
        with nc.named_scope("attn_ff"):
            ff_block(0, hsw_sb)
            for h in range(HPC):
                attention_head(h, kT_sb, vb_sb, qT_sb, ao2_sb, odd_sb)
                ff_block(2 * h + 1, hsw_sb)
                if h < HPC - 1:
                    ff_block(2 * h + 2, hsw_sb)
            nc.sync.dma_start(ao2_sb[64:128, :], odd_sb[:])

        # ---- out^T = Wout_s^T ao + W2_s^T hsw  (shared accumulation) ----
        with nc.named_scope("out"):
            for m in range(D // 128):
                for c in range(NCH):
                    cs = slice(c * NC, (c + 1) * NC)
                    o_ps = psAv.tile([128, NC], F32, tag="av")
                    for kt in range(QI // 128):
                        nc.tensor.matmul(
                            o_ps[:],
                            wout_sb[:, kt * D + m * 128:kt * D + (m + 1) * 128],
                            ao2_sb[:, kt * N:(kt + 1) * N][:, cs],
                            start=(kt == 0), stop=False)
                    for kt in range(KT):
                        nc.tensor.matmul(
                            o_ps[:],
                            w2_sb[:, kt * D + m * 128:kt * D + (m + 1) * 128],
                            hsw_sb[:, kt * N:(kt + 1) * N][:, cs],
                            start=False, stop=(kt == KT - 1))
                    o_sb = outp.tile([128, NC], F32, tag="o")
                    nc.vector.tensor_copy(o_sb[:], o_ps[:])
                    nc.sync.dma_start(T["outT"][m * 128:(m + 1) * 128, :][:, cs],
                                      o_sb[:])
                warmers(1, pool=psFv, tag="ffv")


_NC_CACHE = {}
_LAST_RES = None


def _get_nc(apply_b: bool, use_mask: bool):
    key = (apply_b, use_mask)
    if key not in _NC_CACHE:
        _NC_CACHE[key] = _build(apply_b, use_mask)
    return _NC_CACHE[key]


def kernel(x, context, mask, ln_g, ln_b, cln_g, cln_b, Wq, Wkv, Wout, W1, W2):
    global _LAST_RES
    x = np.asarray(x, np.float32)
    context = np.asarray(context, np.float32)
    mask = np.asarray(mask, np.float32)
    ln_g, ln_b = np.asarray(ln_g, np.float32), np.asarray(ln_b, np.float32)
    cln_g, cln_b = np.asarray(cln_g, np.float32), np.asarray(cln_b, np.float32)
    Wq, Wkv, Wout = (np.asarray(Wq, np.float32), np.asarray(Wkv, np.float32),
                     np.asarray(Wout, np.float32))
    W1, W2 = np.asarray(W1, np.float32), np.asarray(W2, np.float32)

    scale = DH ** -0.5
    use_mask = bool(np.any(mask))
    apply_b = bool(np.any(ln_b) or np.any(cln_b)
                   or np.any(ln_g != 1) or np.any(cln_g != 1))

    xT = [np.ascontiguousarray(x[b].T).astype(np.float16) for b in range(B)]
    cT = [np.ascontiguousarray(context[b].T).astype(np.float16) for b in range(B)]
    mT = [np.ascontiguousarray(mask[b].T).astype(np.float16) for b in range(B)] \
        if use_mask else None
    wkv16 = Wkv.astype(np.float16)
    pack = lambda v: np.ascontiguousarray(v.reshape(KT, 128).T).astype(np.float32)
    gxp, bxp, gcp, bcp_ = pack(ln_g), pack(ln_b), pack(cln_g), pack(cln_b)

    in_maps = []
    for core in range(B * NSH):
        bi, s = core // NSH, core % NSH
        m = {
            "xT": xT[bi],
            "cT": cT[bi],
            "wq": np.ascontiguousarray(
                Wq[:, s * QI:(s + 1) * QI] * scale).astype(np.float16),
            "wkv": wkv16,
            "wout": np.ascontiguousarray(Wout[s * QI:(s + 1) * QI, :]).astype(np.float16),
            "w1": np.ascontiguousarray(np.concatenate(
                [W1[:, s * FFS:(s + 1) * FFS],
                 W1[:, FF + s * FFS:FF + (s + 1) * FFS]], axis=1)).astype(np.float16),
            "w2": np.ascontiguousarray(W2[s * FFS:(s + 1) * FFS, :]).astype(np.float16),
            "gx": gxp, "bx": bxp, "gc": gcp, "bc": bcp_,
        }
        if use_mask:
            m["maskT"] = mT[bi]
        in_maps.append(m)

    nc = _get_nc(apply_b, use_mask)
    res = run_bass_kernel_spmd(nc, in_maps, core_ids=list(range(B * NSH)))
    _LAST_RES = res

    out = np.zeros((B, N, D), np.float32)
    for core in range(B * NSH):
        out[core // NSH] += res.results[core]["outT"].T
    return out
